# revision 5
# baseline (speedup 1.0000x reference)
"""Trainium2 Bass kernel for nn_BinaryLoss (BCE triangle-mesh loss).

Structure
---------
Host (integer combinatorics on the tiny index tensors only): sorted-triangle
key table -> unique keys; undirected GT edge set; per-vertex unique-triangle
counts; candidate-triple membership gt_mask [N,256]; manifold row mask w [N];
edge mask gm [N,16].  Identities used:
  * gt_labels_masked == gt_mask,
  * softplus(x) = -ln(sigmoid(-x)), so every loss term is a log of a product
    of sigmoids.  The device ships segmented PRODUCTS of sigmoids; the host
    takes logs of the few partial products and does the scalar reduction.
    Only the Sigmoid activation is used on device -> one ACT table load.

Device (8 cores data-parallel, per core, logit math in fp16):
  * s = sigmoid(-gsel) maps "2nd/3rd largest logit" to "2nd/3rd SMALLEST
    sigmoid", where fp16 keeps fine relative precision.  Groups are stored
    column-major [128, 16 elems, 50 groups] so the mask passes are plain
    tensor_tensor ops with an outer-broadcast operand (2x fp16 perf mode);
    the three rank extractions are strided segmented tensor_reduce(min).
    7 DVE instructions total for 6400 groups.
  * sp(-p2) = -ln(1 - m2), sp(p3) = -ln(m3): segmented products of (1-m2),
    m3 and sigmoid(-x) ship as [128,48] f32; host does -sum(log(...)).
  * The sp-products half of the output DMAs out early (mid-chain) and two
    heartbeat DMAs ride the chain, keeping the HW DMA engines out of their
    multi-microsecond idle-poll state so the final DMA completes quickly.
Pad rows use -15 (sigmoid -> 1.0 exactly, neutral in products); pad groups
[+8, +7, -15 x14] keep distinct fp16 top-2 so the masked-min chain yields
(near-)neutral pos/neg terms.
"""
import numpy as np

N_CORES = 8
B_PAD = 15.0
NROW = 128           # selected rows per core (one per partition)
M = 256              # logits per row
G = 50               # groups per partition per core (column-major)
GPC = G * 128        # groups per core
L = 8                # max gt_mask nonzeros per row


# ---------------------------------------------------------------- host prep
def _host_prep(pred_logits, points, knn_indices, gt_triangles):
    N, K = knn_indices.shape
    m = (K - 1) * (K - 1)
    num_pts = points.shape[0]
    P = num_pts + 1

    tri = np.sort(np.asarray(gt_triangles, dtype=np.int64), axis=1)
    keys = tri[:, 0] * (P * P) + tri[:, 1] * P + tri[:, 2]
    uk = np.unique(keys)

    ut0, ut1, ut2 = uk // (P * P), (uk // P) % P, uk % P
    counts = np.zeros(P, np.float64)
    np.add.at(counts, ut0, 1.0)
    np.add.at(counts, ut1, (ut1 != ut0).astype(np.float64))
    np.add.at(counts, ut2, (ut2 != ut1).astype(np.float64))
    all_N_gt = counts[np.asarray(knn_indices[:, 0], dtype=np.int64)]

    e_u = np.concatenate([np.minimum(tri[:, 0], tri[:, 1]),
                          np.minimum(tri[:, 1], tri[:, 2]),
                          np.minimum(tri[:, 0], tri[:, 2])])
    e_v = np.concatenate([np.maximum(tri[:, 0], tri[:, 1]),
                          np.maximum(tri[:, 1], tri[:, 2]),
                          np.maximum(tri[:, 0], tri[:, 2])])
    ekeys = np.unique(e_u * P + e_v)

    c = np.asarray(knn_indices[:, 0], dtype=np.int64)[:, None]
    a = np.asarray(knn_indices[:, 1:], dtype=np.int64)
    q = np.minimum(c, a) * P + np.maximum(c, a)
    pos = np.clip(np.searchsorted(ekeys, q.ravel()), 0, len(ekeys) - 1)
    gm = (ekeys[pos] == q.ravel()).reshape(N, K - 1)

    e0 = np.repeat(a, K - 1, axis=1)
    e1 = np.tile(a, (1, K - 1))
    v0 = np.broadcast_to(c, e0.shape)
    cand = np.stack([v0, e0, e1], axis=-1)
    cand.sort(axis=-1)
    ck = cand[..., 0] * (P * P) + cand[..., 1] * P + cand[..., 2]
    cpos = np.clip(np.searchsorted(uk, ck.ravel()), 0, len(uk) - 1)
    gt_mask = (uk[cpos] == ck.ravel()).reshape(N, m)

    all_N_pred = gt_mask.sum(1).astype(np.float64)
    manifold = (all_N_gt * 2.0) == all_N_pred
    w = manifold.astype(np.float32)

    inv_denom = 1.0 / max(float(w.sum(dtype=np.float64)) * m, 1.0)
    inv_cnt = 1.0 / max(float(gm.sum(dtype=np.float64)), 1.0)
    return gt_mask, gm, w, inv_denom, inv_cnt


def _make_shards(x, gt_mask, gm, w):
    """Per-core input dicts: gsel column-major [128, 16*G] fp16 and
    xrows+xm [128, M+L] fp16."""
    N = x.shape[0]
    f16 = np.float16

    sel = np.nonzero(w)[0]
    CAP = NROW * N_CORES
    assert len(sel) <= CAP, (len(sel), CAP)
    xs = np.full((CAP, M), -B_PAD, np.float32)
    xs[:len(sel)] = x[sel]

    rr, cc = np.nonzero(gt_mask[sel])
    row_starts = np.zeros(CAP + 1, np.int64)
    np.add.at(row_starts, rr + 1, 1)
    row_starts = np.cumsum(row_starts)
    ranks = np.arange(len(rr)) - row_starts[rr]
    assert ranks.max(initial=0) < L
    xmv = np.zeros((CAP, L), np.float32)
    xmv[rr, ranks] = xs[rr, cc]

    gn, gi = np.nonzero(gm)
    total = len(gn)
    assert total <= GPC * N_CORES, total
    pl3 = x.reshape(N, 16, 16)
    pad_group = np.full(16, -B_PAD, np.float32)
    pad_group[0] = 8.0
    pad_group[1] = 7.0
    groups = np.broadcast_to(pad_group, (GPC * N_CORES, 16)).copy()
    groups[:total] = pl3[gn, gi, :]

    in_maps = []
    for core in range(N_CORES):
        gsl = groups[core * GPC:(core + 1) * GPC]          # [GPC, 16]
        # group j = g*128 + p  ->  partition p, column e*G + g
        gcm = np.ascontiguousarray(
            gsl.reshape(G, 128, 16).transpose(1, 2, 0)).reshape(128, 16 * G)
        r0 = core * NROW
        xx = np.concatenate([xs[r0:r0 + NROW], xmv[r0:r0 + NROW]], axis=1)
        in_maps.append({"gsel": gcm.astype(f16),
                        "xx": np.ascontiguousarray(xx.astype(f16))})
    return in_maps


# ---------------------------------------------------------------- bass build
def _build_bass():
    from contextlib import ExitStack

    import concourse.bacc as bacc
    import concourse.mybir as mybir
    import concourse.tile as tile

    f32 = mybir.dt.float32
    f16 = mybir.dt.float16
    AFT = mybir.ActivationFunctionType
    ALU = mybir.AluOpType
    AX = mybir.AxisListType

    G16 = G * 16
    SPSEG = M // 8             # 32 sigmoid-product segments per row
    PSEG = G // 10             # 5 pos/neg product segments (10 groups each)

    nc = bacc.Bacc(
        "TRN2", target_bir_lowering=False, debug=False,
        enable_asserts=False, num_devices=N_CORES,
    )
    g_d = nc.dram_tensor("gsel", [128, G16], f16, kind="ExternalInput").ap()
    x_d = nc.dram_tensor("xx", [128, M + L], f16, kind="ExternalInput").ap()
    out_d = nc.dram_tensor("out", [128, 48], f32, kind="ExternalOutput").ap()

    with tile.TileContext(nc) as tc, ExitStack() as ctx:
        from concourse.tile import add_dep_helper

        def order(a_, b_):
            add_dep_helper(b_.ins, a_.ins, sync=True, reason="engine order")

        pool = ctx.enter_context(tc.tile_pool(name="main", bufs=1))

        gt = pool.tile([128, G16], f16)
        nc.sync.dma_start(gt[:], g_d[:, :])
        xt = pool.tile([128, M + L], f16)
        nc.sync.dma_start(xt[:], x_d[:, :])

        acts = []
        # s = sigmoid(-gsel)  (column-major [p, 16, G])
        sgs = pool.tile([128, G16], f16)
        acts.append(nc.scalar.activation(sgs[:], gt[:], AFT.Sigmoid,
                                         scale=-1.0))
        # sigmoid(-x) over selected rows
        sgx = pool.tile([128, M], f16)
        acts.append(nc.scalar.activation(sgx[:], xt[:, :M], AFT.Sigmoid,
                                         scale=-1.0))
        out_t = pool.tile([128, 48], f32)
        # xm sum via activation accumulate
        xm_scr = pool.tile([128, L], f32)
        acts.append(nc.scalar.activation(xm_scr[:], xt[:, M:], AFT.Identity,
                                         accum_out=out_t[:, 42:43]))
        nc.vector.memset(out_t[:, 43:48], 0.0)

        c3 = sgs[:].rearrange("p (e g) -> p e g", g=G)       # [p,16,G]
        cseg = sgs[:].rearrange("p (e g) -> p g e", g=G)     # [p,G,16] strided

        # early product: sp over selected rows
        nc.vector.tensor_reduce(
            out_t[:, 0:SPSEG], sgx[:].rearrange("p (k l) -> p k l", l=8),
            axis=AX.X, op=ALU.mult)
        dma_p1 = nc.sync.dma_start(out_d[:, 0:SPSEG], out_t[:, 0:SPSEG])

        # ---- masked bottom-3 chain ----
        m1 = pool.tile([128, G], f16)
        nc.vector.tensor_reduce(m1[:], cseg, axis=AX.X, op=ALU.min)
        m1b = m1[:].unsqueeze(1).broadcast_to([128, 16, G])
        e1 = pool.tile([128, G16], f16)
        e1_3 = e1[:].rearrange("p (e g) -> p e g", g=G)
        nc.vector.tensor_tensor(e1_3, c3, m1b, ALU.is_le)
        s2 = pool.tile([128, G16], f16)
        nc.vector.tensor_tensor(s2[:], sgs[:], e1[:], ALU.add)
        s2seg = s2[:].rearrange("p (e g) -> p g e", g=G)
        s2_3 = s2[:].rearrange("p (e g) -> p e g", g=G)

        m2 = pool.tile([128, G], f16)
        m2done = nc.vector.tensor_reduce(m2[:], s2seg, axis=AX.X, op=ALU.min)
        m2b = m2[:].unsqueeze(1).broadcast_to([128, 16, G])
        e2 = pool.tile([128, G16], f16)
        e2_3 = e2[:].rearrange("p (e g) -> p e g", g=G)
        nc.vector.tensor_tensor(e2_3, s2_3, m2b, ALU.is_le)
        s3 = pool.tile([128, G16], f16)
        nc.vector.tensor_tensor(s3[:], s2[:], e2[:], ALU.add)
        s3seg = s3[:].rearrange("p (e g) -> p g e", g=G)
        m3 = pool.tile([128, G], f16)
        m3done = nc.vector.tensor_reduce(m3[:], s3seg, axis=AX.X, op=ALU.min)

        # pos term = 1 - m2 on ScalarE
        post = pool.tile([128, G], f16)
        acts.append(nc.scalar.activation(post[:], m2[:], AFT.Identity,
                                         scale=-1.0, bias=1.0))

        # products of 10 -> out
        nc.vector.tensor_reduce(
            out_t[:, 32:32 + PSEG], post[:].rearrange("p (k l) -> p k l", l=10),
            axis=AX.X, op=ALU.mult)
        nc.vector.tensor_reduce(
            out_t[:, 37:37 + PSEG], m3[:].rearrange("p (k l) -> p k l", l=10),
            axis=AX.X, op=ALU.mult)
        nc.sync.dma_start(out_d[:, 32:48], out_t[:, 32:48],
                          single_packet=True)

        # heartbeats: keep DMA engines awake through the chain
        hb = pool.tile([128, 4], f16)
        hb1 = nc.sync.dma_start(hb[:, 0:2], g_d[:, 0:2])
        add_dep_helper(hb1.ins, m2done.ins, sync=True, reason="hb spacing")
        hb2 = nc.sync.dma_start(hb[:, 2:4], g_d[:, 0:2])
        add_dep_helper(hb2.ins, m3done.ins, sync=True, reason="hb spacing")

        for a_, b_ in zip(acts, acts[1:]):
            order(a_, b_)

    nc.compile()
    return nc


# ---------------------------------------------------------------- entrypoint
def _run(pred_logits, points, knn_indices, gt_triangles, **run_kwargs):
    from concourse.bass_utils import run_bass_kernel_spmd

    x = np.ascontiguousarray(np.asarray(pred_logits, dtype=np.float32))
    gt_mask, gm, w, inv_denom, inv_cnt = _host_prep(
        pred_logits, points, knn_indices, gt_triangles)
    in_maps = _make_shards(x, gt_mask, gm, w)
    nc = _build_bass()
    res = run_bass_kernel_spmd(nc, in_maps, core_ids=list(range(N_CORES)),
                               **run_kwargs)
    S_sp = S_xm = S_pos = S_neg = 0.0
    for r in res.results:
        o = np.asarray(r["out"], dtype=np.float64).reshape(128, 48)
        S_sp += -np.log(o[:, 0:32]).sum()
        S_pos += -np.log(o[:, 32:37]).sum()
        S_neg += -np.log(o[:, 37:42]).sum()
        S_xm += o[:, 42].sum()
    total = np.array([(S_sp - S_xm) * inv_denom,
                      S_pos * inv_cnt,
                      S_neg * inv_cnt])
    return total.astype(np.float32), res


def kernel(pred_logits, points, knn_indices, gt_triangles):
    out, _ = _run(pred_logits, points, knn_indices, gt_triangles)
    return out


# revision 7
# speedup vs baseline: 1.1344x; 1.1344x over previous
"""Trainium2 Bass kernel for nn_BinaryLoss (BCE triangle-mesh loss).

Structure
---------
Host (integer combinatorics on the tiny index tensors only): sorted-triangle
key table -> unique keys; undirected GT edge set; per-vertex unique-triangle
counts; candidate-triple membership gt_mask [N,256]; manifold row mask w [N];
edge mask gm [N,16].  Identities used:
  * gt_labels_masked == gt_mask,
  * softplus(x) = -ln(sigmoid(-x)), so every loss term is a log of a product
    of sigmoids.  The device ships segmented PRODUCTS of sigmoids; the host
    takes logs of the few partial products and does the scalar reduction.
    Only the Sigmoid activation is used on device -> one ACT table load.

Device (8 cores data-parallel, per core, logit math in fp16):
  * s = sigmoid(-gsel) maps "2nd/3rd largest logit" to "2nd/3rd SMALLEST
    sigmoid", where fp16 keeps fine relative precision.  Groups are stored
    column-major [128, 16 elems, 50 groups] so the mask passes are plain
    tensor_tensor ops with an outer-broadcast operand (2x fp16 perf mode);
    the three rank extractions are strided segmented tensor_reduce(min).
    7 DVE instructions total for 6400 groups.
  * sp(-p2) = -ln(1 - m2), sp(p3) = -ln(m3): segmented products of (1-m2),
    m3 and sigmoid(-x) ship as [128,48] f32; host does -sum(log(...)).
  * The sp-products half of the output DMAs out early (mid-chain) and two
    heartbeat DMAs ride the chain, keeping the HW DMA engines out of their
    multi-microsecond idle-poll state so the final DMA completes quickly.
Pad rows use -15 (sigmoid -> 1.0 exactly, neutral in products); pad groups
[+8, +7, -15 x14] keep distinct fp16 top-2 so the masked-min chain yields
(near-)neutral pos/neg terms.
"""
import numpy as np

N_CORES = 8
B_PAD = 15.0
NROW = 128           # selected rows per core (one per partition)
M = 256              # logits per row
G = 50               # groups per partition per core (column-major)
GPC = G * 128        # groups per core
L = 8                # max gt_mask nonzeros per row


# ---------------------------------------------------------------- host prep
def _host_prep(pred_logits, points, knn_indices, gt_triangles):
    N, K = knn_indices.shape
    m = (K - 1) * (K - 1)
    num_pts = points.shape[0]
    P = num_pts + 1

    tri = np.sort(np.asarray(gt_triangles, dtype=np.int64), axis=1)
    keys = tri[:, 0] * (P * P) + tri[:, 1] * P + tri[:, 2]
    uk = np.unique(keys)

    ut0, ut1, ut2 = uk // (P * P), (uk // P) % P, uk % P
    counts = np.zeros(P, np.float64)
    np.add.at(counts, ut0, 1.0)
    np.add.at(counts, ut1, (ut1 != ut0).astype(np.float64))
    np.add.at(counts, ut2, (ut2 != ut1).astype(np.float64))
    all_N_gt = counts[np.asarray(knn_indices[:, 0], dtype=np.int64)]

    e_u = np.concatenate([np.minimum(tri[:, 0], tri[:, 1]),
                          np.minimum(tri[:, 1], tri[:, 2]),
                          np.minimum(tri[:, 0], tri[:, 2])])
    e_v = np.concatenate([np.maximum(tri[:, 0], tri[:, 1]),
                          np.maximum(tri[:, 1], tri[:, 2]),
                          np.maximum(tri[:, 0], tri[:, 2])])
    ekeys = np.unique(e_u * P + e_v)

    c = np.asarray(knn_indices[:, 0], dtype=np.int64)[:, None]
    a = np.asarray(knn_indices[:, 1:], dtype=np.int64)
    q = np.minimum(c, a) * P + np.maximum(c, a)
    pos = np.clip(np.searchsorted(ekeys, q.ravel()), 0, len(ekeys) - 1)
    gm = (ekeys[pos] == q.ravel()).reshape(N, K - 1)

    e0 = np.repeat(a, K - 1, axis=1)
    e1 = np.tile(a, (1, K - 1))
    v0 = np.broadcast_to(c, e0.shape)
    cand = np.stack([v0, e0, e1], axis=-1)
    cand.sort(axis=-1)
    ck = cand[..., 0] * (P * P) + cand[..., 1] * P + cand[..., 2]
    cpos = np.clip(np.searchsorted(uk, ck.ravel()), 0, len(uk) - 1)
    gt_mask = (uk[cpos] == ck.ravel()).reshape(N, m)

    all_N_pred = gt_mask.sum(1).astype(np.float64)
    manifold = (all_N_gt * 2.0) == all_N_pred
    w = manifold.astype(np.float32)

    inv_denom = 1.0 / max(float(w.sum(dtype=np.float64)) * m, 1.0)
    inv_cnt = 1.0 / max(float(gm.sum(dtype=np.float64)), 1.0)
    return gt_mask, gm, w, inv_denom, inv_cnt


def _make_shards(x, gt_mask, gm, w):
    """Per-core input dicts: gsel column-major [128, 16*G] fp16 and
    xrows+xm [128, M+L] fp16."""
    N = x.shape[0]
    f16 = np.float16

    sel = np.nonzero(w)[0]
    CAP = NROW * N_CORES
    assert len(sel) <= CAP, (len(sel), CAP)
    xs = np.full((CAP, M), -B_PAD, np.float32)
    xs[:len(sel)] = x[sel]

    rr, cc = np.nonzero(gt_mask[sel])
    row_starts = np.zeros(CAP + 1, np.int64)
    np.add.at(row_starts, rr + 1, 1)
    row_starts = np.cumsum(row_starts)
    ranks = np.arange(len(rr)) - row_starts[rr]
    assert ranks.max(initial=0) < L
    xmv = np.zeros((CAP, L), np.float32)
    xmv[rr, ranks] = xs[rr, cc]

    gn, gi = np.nonzero(gm)
    total = len(gn)
    assert total <= GPC * N_CORES, total
    pl3 = x.reshape(N, 16, 16)
    pad_group = np.full(16, -B_PAD, np.float32)
    pad_group[0] = 8.0
    pad_group[1] = 7.0
    groups = np.broadcast_to(pad_group, (GPC * N_CORES, 16)).copy()
    groups[:total] = pl3[gn, gi, :]

    in_maps = []
    for core in range(N_CORES):
        gsl = groups[core * GPC:(core + 1) * GPC]          # [GPC, 16]
        # group j = g*128 + p  ->  partition p, column e*G + g
        gcm = np.ascontiguousarray(
            gsl.reshape(G, 128, 16).transpose(1, 2, 0)).reshape(128, 16 * G)
        r0 = core * NROW
        xx = np.concatenate([xs[r0:r0 + NROW], xmv[r0:r0 + NROW]], axis=1)
        in_maps.append({"gsel": gcm.astype(f16),
                        "xx": np.ascontiguousarray(xx.astype(f16))})
    return in_maps


# ---------------------------------------------------------------- bass build
def _build_bass():
    from contextlib import ExitStack

    import concourse.bacc as bacc
    import concourse.mybir as mybir
    import concourse.tile as tile

    f32 = mybir.dt.float32
    f16 = mybir.dt.float16
    AFT = mybir.ActivationFunctionType
    ALU = mybir.AluOpType
    AX = mybir.AxisListType

    G16 = G * 16
    SPSEG = M // 8             # 32 sigmoid-product segments per row
    PSEG = G // 10             # 5 pos/neg product segments (10 groups each)

    nc = bacc.Bacc(
        "TRN2", target_bir_lowering=False, debug=False,
        enable_asserts=False, num_devices=N_CORES,
    )
    g_d = nc.dram_tensor("gsel", [128, G16], f16, kind="ExternalInput").ap()
    x_d = nc.dram_tensor("xx", [128, M + L], f16, kind="ExternalInput").ap()
    out_d = nc.dram_tensor("out", [128, 48], f32, kind="ExternalOutput").ap()

    with tile.TileContext(nc) as tc, ExitStack() as ctx:
        from concourse.tile import add_dep_helper

        def order(a_, b_):
            add_dep_helper(b_.ins, a_.ins, sync=True, reason="engine order")

        pool = ctx.enter_context(tc.tile_pool(name="main", bufs=1))

        gt = pool.tile([128, G16], f16)
        nc.sync.dma_start(gt[:], g_d[:, :])
        xt = pool.tile([128, M + L], f16)
        nc.sync.dma_start(xt[:], x_d[:, :])

        acts = []
        # s = sigmoid(-gsel)  (column-major [p, 16, G])
        sgs = pool.tile([128, G16], f16)
        acts.append(nc.scalar.activation(sgs[:], gt[:], AFT.Sigmoid,
                                         scale=-1.0))
        # sigmoid(-x) over selected rows
        sgx = pool.tile([128, M], f16)
        acts.append(nc.scalar.activation(sgx[:], xt[:, :M], AFT.Sigmoid,
                                         scale=-1.0))
        out_t = pool.tile([128, 48], f32)
        # xm sum via activation accumulate
        xm_scr = pool.tile([128, L], f32)
        acts.append(nc.scalar.activation(xm_scr[:], xt[:, M:], AFT.Identity,
                                         accum_out=out_t[:, 42:43]))
        nc.vector.memset(out_t[:, 43:48], 0.0)

        c3 = sgs[:].rearrange("p (e g) -> p e g", g=G)       # [p,16,G]

        # early product: sp over selected rows
        nc.vector.tensor_reduce(
            out_t[:, 0:SPSEG], sgx[:].rearrange("p (k l) -> p k l", l=8),
            axis=AX.X, op=ALU.mult)
        dma_p1 = nc.sync.dma_start(out_d[:, 0:SPSEG], out_t[:, 0:SPSEG])

        def min_tree(src, scratch_name):
            """min over the 16 rows of a [p, 16*G] column-major tile via
            4 tensor_tensor stages (fp16 2x perf mode)."""
            t8 = pool.tile([128, 8 * G], f16, name=f"{scratch_name}8",
                           tag=f"{scratch_name}8")
            nc.vector.tensor_tensor(t8[:], src[:, :8 * G], src[:, 8 * G:],
                                    ALU.min)
            t4 = pool.tile([128, 4 * G], f16, name=f"{scratch_name}4",
                           tag=f"{scratch_name}4")
            nc.vector.tensor_tensor(t4[:], t8[:, :4 * G], t8[:, 4 * G:],
                                    ALU.min)
            t2 = pool.tile([128, 2 * G], f16, name=f"{scratch_name}2",
                           tag=f"{scratch_name}2")
            nc.vector.tensor_tensor(t2[:], t4[:, :2 * G], t4[:, 2 * G:],
                                    ALU.min)
            m = pool.tile([128, G], f16, name=f"{scratch_name}1",
                          tag=f"{scratch_name}1")
            done = nc.vector.tensor_tensor(m[:], t2[:, :G], t2[:, G:],
                                           ALU.min)
            return m, done

        # ---- masked bottom-3 chain ----
        m1, _ = min_tree(sgs, "m1t")
        m1b = m1[:].unsqueeze(1).broadcast_to([128, 16, G])
        e1 = pool.tile([128, G16], f16)
        e1_3 = e1[:].rearrange("p (e g) -> p e g", g=G)
        nc.vector.tensor_tensor(e1_3, c3, m1b, ALU.is_le)
        s2 = pool.tile([128, G16], f16)
        nc.vector.tensor_tensor(s2[:], sgs[:], e1[:], ALU.add)
        s2_3 = s2[:].rearrange("p (e g) -> p e g", g=G)

        m2, m2done = min_tree(s2, "m2t")
        m2b = m2[:].unsqueeze(1).broadcast_to([128, 16, G])
        e2 = pool.tile([128, G16], f16)
        e2_3 = e2[:].rearrange("p (e g) -> p e g", g=G)
        e2done = nc.vector.tensor_tensor(e2_3, s2_3, m2b, ALU.is_le)
        s3 = pool.tile([128, G16], f16)
        nc.vector.tensor_tensor(s3[:], s2[:], e2[:], ALU.add)
        m3, m3done = min_tree(s3, "m3t")

        # pos term = 1 - m2 on ScalarE
        post = pool.tile([128, G], f16)
        acts.append(nc.scalar.activation(post[:], m2[:], AFT.Identity,
                                         scale=-1.0, bias=1.0))

        # products of 10 -> out
        nc.vector.tensor_reduce(
            out_t[:, 32:32 + PSEG], post[:].rearrange("p (k l) -> p k l", l=10),
            axis=AX.X, op=ALU.mult)
        nc.vector.tensor_reduce(
            out_t[:, 37:37 + PSEG], m3[:].rearrange("p (k l) -> p k l", l=10),
            axis=AX.X, op=ALU.mult)
        nc.sync.dma_start(out_d[:, 32:48], out_t[:, 32:48],
                          single_packet=True)

        # heartbeats: keep DMA engines awake through the chain
        hb = pool.tile([128, 6], f16)
        hb1 = nc.sync.dma_start(hb[:, 0:2], g_d[:, 0:2])
        add_dep_helper(hb1.ins, m2done.ins, sync=True, reason="hb spacing")
        hb2 = nc.sync.dma_start(hb[:, 2:4], g_d[:, 0:2])
        add_dep_helper(hb2.ins, e2done.ins, sync=True, reason="hb spacing")
        hb3 = nc.sync.dma_start(hb[:, 4:6], g_d[:, 0:2])
        add_dep_helper(hb3.ins, m3done.ins, sync=True, reason="hb spacing")

        for a_, b_ in zip(acts, acts[1:]):
            order(a_, b_)

    nc.compile()
    return nc


# ---------------------------------------------------------------- entrypoint
def _run(pred_logits, points, knn_indices, gt_triangles, **run_kwargs):
    from concourse.bass_utils import run_bass_kernel_spmd

    x = np.ascontiguousarray(np.asarray(pred_logits, dtype=np.float32))
    gt_mask, gm, w, inv_denom, inv_cnt = _host_prep(
        pred_logits, points, knn_indices, gt_triangles)
    in_maps = _make_shards(x, gt_mask, gm, w)
    nc = _build_bass()
    res = run_bass_kernel_spmd(nc, in_maps, core_ids=list(range(N_CORES)),
                               **run_kwargs)
    S_sp = S_xm = S_pos = S_neg = 0.0
    for r in res.results:
        o = np.asarray(r["out"], dtype=np.float64).reshape(128, 48)
        S_sp += -np.log(o[:, 0:32]).sum()
        S_pos += -np.log(o[:, 32:37]).sum()
        S_neg += -np.log(o[:, 37:42]).sum()
        S_xm += o[:, 42].sum()
    total = np.array([(S_sp - S_xm) * inv_denom,
                      S_pos * inv_cnt,
                      S_neg * inv_cnt])
    return total.astype(np.float32), res


def kernel(pred_logits, points, knn_indices, gt_triangles):
    out, _ = _run(pred_logits, points, knn_indices, gt_triangles)
    return out
